# revision 4
# baseline (speedup 1.0000x reference)
"""GNN message-passing kernel for 8 trn2 NeuronCores (Bass/Tile).

Algorithm (reference):
    A = x @ W_interact[:128] + b_interact          # [N,128]
    B = x @ W_interact[128:]                       # [N,128]
    m_i = segment_sum(relu(A[src] + B[dst]), src) / 4
    out = x + relu((x + m_i) @ W_update + b_update)

Sharding: nodes (and their outgoing edges, keyed by src) are split across 8
cores in contiguous ranges of 6250. Every core computes the full B table
(needed for arbitrary dst) and its own A slice on-device, then processes its
edges in 49 node-blocks of 128. Per 128-edge tile: gather A[src]/B[dst] rows
with batched dma_gather, relu(A+B) on DVE, and a one-hot matmul accumulates
the segment-sum into PSUM. All cores run ONE program (SPMD), so the per-block
tile counts are padded to uniform constants derived from the input.
"""
import numpy as np

N = 50000
E = 800000
H = 128
NCORES = 8
NPC = N // NCORES          # nodes per core (6250)
NBLK = 49                  # 128-node blocks per core (49*128 = 6272)
NPAD = NBLK * 128          # padded nodes per core
BSPLIT = 32768             # B table split point (int16 index limit)
NTOT = NCORES * NPAD       # padded total rows of B table (50176)


def _wrap_idx(flat_i16):
    """dma_gather index layout: idx j -> partition j%16, col j//16, x8 replicas."""
    a = flat_i16.reshape(-1, 16).T  # [16, n/16]
    return np.ascontiguousarray(np.tile(a, (8, 1)))


def _prep(edge_index):
    """Partition+pad edges into the uniform (core, block, class) tile grid."""
    src = np.asarray(edge_index[0], dtype=np.int64)
    dst = np.asarray(edge_index[1], dtype=np.int64)
    order = np.argsort(src, kind="stable")
    src = src[order]
    dst = dst[order]

    # per (core, block) edge runs
    blk_of = src // 128                      # global block id, 0..390 (since NPC=6250, block 48 of a core spans 2 cores? no:)
    # NOTE: cores own node ranges of 6250 which is NOT a multiple of 128.
    # Define per-core local blocks: local = src - c*6250, block = local//128.
    core_of = src // NPC
    local = src - core_of * NPC
    lblk = local // 128

    # remap dst into the padded B-table row space: node n -> (n//NPC)*NPAD + n%NPC
    dstp = (dst // NPC) * NPAD + dst % NPC

    # count edges per (core, block, class)
    cls = (dstp >= BSPLIT).astype(np.int64)
    key = (core_of * NBLK + lblk) * 2 + cls
    counts = np.bincount(key, minlength=NCORES * NBLK * 2).reshape(NCORES, NBLK, 2)
    K0 = int(np.ceil(counts[:, :, 0].max() / 128))
    K1 = int(np.ceil(counts[:, :, 1].max() / 128))
    K0 = max(K0, 1)
    K1 = max(K1, 1)
    T = NBLK * (K0 + K1)

    # fill tile arrays
    src_cmp = np.full((NCORES, T * 128), -1.0, dtype=np.float32)
    idxA = np.zeros((NCORES, T * 128), dtype=np.int16)
    idxB = np.zeros((NCORES, T * 128), dtype=np.int16)

    # edges are sorted by src; group-split by key
    order2 = np.argsort(key, kind="stable")
    s2, d2, k2 = src[order2], dstp[order2], key[order2]
    starts = np.searchsorted(k2, np.arange(NCORES * NBLK * 2))
    ends = np.searchsorted(k2, np.arange(NCORES * NBLK * 2) + 1)
    for c in range(NCORES):
        for b in range(NBLK):
            base = b * (K0 + K1) * 128
            for cl, K, off in ((0, K0, 0), (1, K1, K0 * 128)):
                kk = (c * NBLK + b) * 2 + cl
                st, en = starts[kk], ends[kk]
                n = en - st
                if n == 0:
                    continue
                sl = slice(base + off, base + off + n)
                src_cmp[c, sl] = (s2[st:en] - (c * NPC + b * 128)).astype(np.float32)
                idxA[c, sl] = (s2[st:en] - c * NPC).astype(np.int16)
                dd = d2[st:en]
                idxB[c, sl] = (dd - (BSPLIT if cl else 0)).astype(np.int16)
    return K0, K1, T, src_cmp, idxA, idxB


def _build(K0, K1, T):
    from concourse import bass, bacc, mybir
    import concourse.tile as tile
    from concourse.masks import make_identity

    KT = K0 + K1
    nc = bacc.Bacc("TRN2", target_bir_lowering=False, debug=False)
    f32, i16 = mybir.dt.float32, mybir.dt.int16

    xT_t = nc.dram_tensor("xT", [128, NTOT], f32, kind="ExternalInput")
    xown_t = nc.dram_tensor("xown", [NPAD, H], f32, kind="ExternalInput")
    w1a_t = nc.dram_tensor("w1a", [H, H], f32, kind="ExternalInput")
    w1b_t = nc.dram_tensor("w1b", [H, H], f32, kind="ExternalInput")
    wu_t = nc.dram_tensor("wu", [H, H], f32, kind="ExternalInput")
    bi_t = nc.dram_tensor("bi", [1, H], f32, kind="ExternalInput")
    bu_t = nc.dram_tensor("bu", [1, H], f32, kind="ExternalInput")
    scmp_t = nc.dram_tensor("scmp", [128, T], f32, kind="ExternalInput")
    idxA_t = nc.dram_tensor("idxA", [128, T * 8], i16, kind="ExternalInput")
    idxB_t = nc.dram_tensor("idxB", [128, T * 8], i16, kind="ExternalInput")
    out_t = nc.dram_tensor("out", [NPAD, H], f32, kind="ExternalOutput")

    B_d = nc.dram_tensor("Btab", [NTOT, H], f32)
    A_d = nc.dram_tensor("Atab", [NPAD, H], f32)

    iota_np = np.tile(np.arange(128, dtype=np.float32), (128, 1))
    iota_d = nc.inline_tensor(iota_np, name="iota")
    ones_d = nc.inline_tensor(np.ones((1, 128), np.float32), name="ones1")

    with tile.TileContext(nc) as tc:
        with tc.tile_pool(name="w", bufs=1) as wp, \
             tc.tile_pool(name="sb", bufs=3) as sp, \
             tc.tile_pool(name="va", bufs=3) as vap, \
             tc.tile_pool(name="vb", bufs=3) as vbp, \
             tc.tile_pool(name="ps", bufs=2, space="PSUM") as pp, \
             tc.tile_pool(name="ms", bufs=2, space="PSUM") as mp:
            # --- constants / weights ---
            w1a = wp.tile([H, H], f32, tag="w1a")
            nc.sync.dma_start(out=w1a[:], in_=w1a_t[:, :])
            w1b = wp.tile([H, H], f32, tag="w1b")
            nc.sync.dma_start(out=w1b[:], in_=w1b_t[:, :])
            wu = wp.tile([H, H], f32, tag="wu")
            nc.sync.dma_start(out=wu[:], in_=wu_t[:, :])
            iota = wp.tile([128, 128], f32, tag="iota")
            nc.sync.dma_start(out=iota[:], in_=iota_d[:, :])
            ones1 = wp.tile([1, 128], f32, tag="ones1")
            nc.sync.dma_start(out=ones1[:], in_=ones_d[:, :])
            ident = wp.tile([128, 128], f32, tag="ident")
            make_identity(nc, ident[:])
            bi_row = wp.tile([1, 128], f32, tag="bi_row")
            nc.sync.dma_start(out=bi_row[:], in_=bi_t[:, :])
            bu_row = wp.tile([1, 128], f32, tag="bu_row")
            nc.sync.dma_start(out=bu_row[:], in_=bu_t[:, :])
            # broadcast biases across partitions via ones-matmul
            bi_ps = pp.tile([128, 128], f32, tag="pps")
            nc.tensor.matmul(out=bi_ps[:], lhsT=ones1[:], rhs=bi_row[:],
                             start=True, stop=True)
            bi_bc = wp.tile([128, 128], f32, tag="bi_bc")
            nc.vector.tensor_copy(bi_bc[:], bi_ps[:])
            bu_ps = pp.tile([128, 128], f32, tag="pps")
            nc.tensor.matmul(out=bu_ps[:], lhsT=ones1[:], rhs=bu_row[:],
                             start=True, stop=True)
            bu_bc = wp.tile([128, 128], f32, tag="bu_bc")
            nc.vector.tensor_copy(bu_bc[:], bu_ps[:])

            # edge index arrays resident in SBUF
            scmp = wp.tile([128, T], f32, tag="scmp")
            nc.sync.dma_start(out=scmp[:], in_=scmp_t[:, :])
            idxA = wp.tile([128, T * 8], i16, tag="idxA")
            nc.sync.dma_start(out=idxA[:], in_=idxA_t[:, :])
            idxB = wp.tile([128, T * 8], i16, tag="idxB")
            nc.sync.dma_start(out=idxB[:], in_=idxB_t[:, :])

            # --- phase 1: B table (all nodes) + A table (own nodes) ---
            NCH = NTOT // 128  # 392
            own_lo = 0  # xT columns are global; own slice differs per core -> use partition id? No: SPMD same program, but A differs per core!
            # A is built from xown (per-core input) instead: transpose xown chunks.
            for ch in range(NCH):
                xc = sp.tile([128, 128], f32, tag="xc")
                nc.sync.dma_start(out=xc[:], in_=xT_t[:, ch * 128:(ch + 1) * 128])
                bps = pp.tile([128, 128], f32, tag="pps")
                nc.tensor.matmul(out=bps[:], lhsT=xc[:], rhs=w1b[:],
                                 start=True, stop=True)
                bsb = sp.tile([128, 128], f32, tag="bsb")
                nc.vector.tensor_copy(bsb[:], bps[:])
                nc.sync.dma_start(out=B_d[ch * 128:(ch + 1) * 128, :], in_=bsb[:])
            # A: from xown [NPAD, H] row-major -> transpose each chunk on PE
            for ch in range(NBLK):
                xr = sp.tile([128, 128], f32, tag="xr")
                nc.sync.dma_start(out=xr[:], in_=xown_t[ch * 128:(ch + 1) * 128, :])
                xtp = pp.tile([128, 128], f32, tag="pps")
                nc.tensor.transpose(out=xtp[:], in_=xr[:], identity=ident[:])
                xts = sp.tile([128, 128], f32, tag="xts")
                nc.vector.tensor_copy(xts[:], xtp[:])
                aps = pp.tile([128, 128], f32, tag="pps")
                nc.tensor.matmul(out=aps[:], lhsT=xts[:], rhs=w1a[:],
                                 start=True, stop=True)
                asb = sp.tile([128, 128], f32, tag="asb")
                nc.vector.tensor_add(out=asb[:], in0=aps[:], in1=bi_bc[:])
                nc.sync.dma_start(out=A_d[ch * 128:(ch + 1) * 128, :], in_=asb[:])

            # --- phase 2: edge tiles ---
            def gathers(idx_sb, table_ap, t_lo, n_tiles, tag, pool):
                """Batch (<=8 tiles each) dma_gather calls; returns list of
                (tile_handle, first_tile, ntile)."""
                res = []
                t = t_lo
                left = n_tiles
                while left > 0:
                    nt = min(8, left)
                    g = pool.tile([128, nt, H], f32, tag=tag)
                    ni = nt * 128
                    nc.gpsimd.dma_gather(
                        g[:], table_ap, idx_sb[:, t * 8:(t * 8 + ni // 16)],
                        ni, ni, H)
                    res.append((g, t, nt))
                    t += nt
                    left -= nt
                return res

            for b in range(NBLK):
                t0 = b * KT
                ga = gathers(idxA, A_d[:, :], t0, KT, "va", vap)
                gb0 = gathers(idxB, B_d[0:BSPLIT, :], t0, K0, "vb", vbp)
                gb1 = gathers(idxB, B_d[BSPLIT:NTOT, :], t0 + K0, K1, "vb", vbp)
                m_ps = mp.tile([128, 128], f32, tag="m")

                def tile_slices(glist):
                    out = {}
                    for g, tstart, ntile in glist:
                        for j in range(ntile):
                            out[tstart + j] = g[:, j, :]
                    return out
                va_s = tile_slices(ga)
                vb_s = tile_slices(gb0 + gb1)

                for k in range(KT):
                    t = t0 + k
                    oh = sp.tile([128, 128], f32, tag="oh")
                    nc.vector.tensor_tensor(
                        out=oh[:], in0=scmp[:, t:t + 1].to_broadcast([128, 128]),
                        in1=iota[:], op=mybir.AluOpType.is_equal)
                    vs = sp.tile([128, 128], f32, tag="vs")
                    nc.vector.tensor_add(out=vs[:], in0=va_s[t], in1=vb_s[t])
                    nc.vector.tensor_scalar_max(vs[:], vs[:], 0.0)
                    nc.tensor.matmul(out=m_ps[:], lhsT=oh[:], rhs=vs[:],
                                     start=(k == 0), stop=(k == KT - 1))

                # --- finish block b ---
                xb = sp.tile([128, 128], f32, tag="xb")
                nc.sync.dma_start(out=xb[:], in_=xown_t[b * 128:(b + 1) * 128, :])
                u = sp.tile([128, 128], f32, tag="u")
                nc.vector.tensor_scalar_mul(u[:], m_ps[:], 0.25)
                nc.vector.tensor_add(out=u[:], in0=u[:], in1=xb[:])
                utp = pp.tile([128, 128], f32, tag="pps")
                nc.tensor.transpose(out=utp[:], in_=u[:], identity=ident[:])
                uts = sp.tile([128, 128], f32, tag="uts")
                nc.vector.tensor_copy(uts[:], utp[:])
                zps = pp.tile([128, 128], f32, tag="pps")
                nc.tensor.matmul(out=zps[:], lhsT=uts[:], rhs=wu[:],
                                 start=True, stop=True)
                zs = sp.tile([128, 128], f32, tag="zs")
                nc.vector.tensor_add(out=zs[:], in0=zps[:], in1=bu_bc[:])
                nc.vector.tensor_scalar_max(zs[:], zs[:], 0.0)
                nc.vector.tensor_add(out=zs[:], in0=zs[:], in1=xb[:])
                nc.sync.dma_start(out=out_t[b * 128:(b + 1) * 128, :], in_=zs[:])
    nc.compile()
    return nc


_CACHE = {}


def kernel(x, edge_index, W_interact, b_interact, W_update, b_update):
    from concourse.bass_utils import run_bass_kernel_spmd

    x = np.asarray(x, dtype=np.float32)
    W_interact = np.asarray(W_interact, dtype=np.float32)
    b_interact = np.asarray(b_interact, dtype=np.float32)
    W_update = np.asarray(W_update, dtype=np.float32)
    b_update = np.asarray(b_update, dtype=np.float32)

    K0, K1, T, src_cmp, idxA, idxB = _prep(edge_index)

    key = (K0, K1, T)
    if key not in _CACHE:
        _CACHE[key] = _build(K0, K1, T)
    nc = _CACHE[key]

    # xT padded to NTOT columns (pad rows of x with zeros, per-core 6272 pad)
    xpad = np.zeros((NTOT, H), np.float32)
    for c in range(NCORES):
        xpad[c * NPAD:c * NPAD + NPC] = x[c * NPC:(c + 1) * NPC]
    xT = np.ascontiguousarray(xpad.T)

    in_maps = []
    for c in range(NCORES):
        xown = xpad[c * NPAD:(c + 1) * NPAD]
        in_maps.append({
            "xT": xT,
            "xown": np.ascontiguousarray(xown),
            "w1a": np.ascontiguousarray(W_interact[:H]),
            "w1b": np.ascontiguousarray(W_interact[H:]),
            "wu": np.ascontiguousarray(W_update),
            "bi": b_interact.reshape(1, H),
            "bu": b_update.reshape(1, H),
            "scmp": np.ascontiguousarray(src_cmp[c].reshape(T, 128).T),
            "idxA": _wrap_idx(idxA[c]),
            "idxB": _wrap_idx(idxB[c]),
        })

    res = run_bass_kernel_spmd(nc, in_maps, core_ids=list(range(NCORES)))
    out = np.empty((N, H), np.float32)
    for c in range(NCORES):
        out[c * NPC:(c + 1) * NPC] = res.results[c]["out"][:NPC]
    return out
